# revision 22
# baseline (speedup 1.0000x reference)
"""Trainium2 Bass kernel for 16-head causal MHA (B=2, T=2048, D=1024, fp32 I/O).

Sharding: core c owns batch c//4 and head-quad c%4 (heads 4q..4q+3, as two
head-pairs). It computes Q/K/V projections for its 256 q/k/v dims, causal
attention for its 4 heads, and a partial output [2048, 1024] (bf16); the host
sums the 4 partials per batch in f64. One batch per core halves the partial-
output drain (PSUM->SBUF casts + DMA) and the x^T input DMA vs 2-batch cores.

Per-core device program, matmul inputs bf16 (2 cols/PE-cycle + fast weight
load), fp32 PSUM accumulation. The schedule is emitted as a 4-stage software
pipeline (QK rc-block + V chunk quad + attention qn + norm/out-proj qn) so
the ScalarE exp stream - the bottleneck engine at ~92us - starts a few us in
and stays saturated, while PE/DVE/DMA work (projections, out-proj, casts)
fills the gaps and keeps the PE HAM-warm:
  - Q^T, K^T = W.T @ x^T (weights stationary, N=512 moving blocks)
  - V natural = x @ Wv with a ones column per head (denominator for free)
  - attention in S^T layout per head-pair: the two heads' K=64 contractions
    sit at PE base partitions 0/64 (row-tiled, concurrent); exp on ScalarE
    with the 1/sqrt(dk) scale folded in; causality = skipping fully-masked
    blocks + one [128,1024] staircase multiply per diagonal 2-chunk group
    (mask2 packs the 4 staircase patterns contiguously)
  - normalization: 1/den via DVE reciprocal_approx_fast read straight from
    the PSUM denominator row; broadcast across partitions via a K=1 matmul;
    applied by a fused tensor_mul that also performs the PSUM->SBUF move
  - partial out per 128-query chunk: two accumulating K=128 matmuls (one per
    head-pair), cast to bf16, DMA'd per 1024-col row block.

Infrastructure: the external walrus allows only ONE sync wait per
instruction; a post-pass hoists extra waits onto single-wait no-ops and the
TileContext closing drain is split into a chain of single-wait drains.
"""

import numpy as np

import bass_rust
from bass_rust import ScopedClock
import concourse.bass as bass
import concourse.mybir as mybir
import concourse.tile as tile

F32 = mybir.dt.float32
BF16 = mybir.dt.bfloat16
F32R = BF16
B, T, D = 2, 2048, 1024
NCORES = 8
P = 128          # partitions / feature chunk
FC = D // P      # 8 feature chunks
QW = 512         # query block width (PSUM bank)
QN = T // QW     # 4 query blocks
KC = T // P      # 16 key chunks
DK = 64
CW = 256         # q/k/v dims per core (4 heads x 64)
NPAIR = 2        # head-pairs per core

# Set True to offload half the diagonal-mask multiplies to GpSimd.
GPSIMD_MASKS = False

# ---------------------------------------------------------------------------
# TileContext drain fix: the external walrus in this container allows only ONE
# sync wait per instruction, but Tile's closing drain packs one wait per active
# proc. Split it into a chain of single-wait drains (same semantics).
_PATCHED = False


def _patched_drain_and_barrier(self, tick_clock, wait_clock):
    nc = self.nc
    drain_inst = nc.sync.drain()
    wait_clock.add_sem_waits(
        drain_inst.ins, ScopedClock({None: tick_clock.global_clock})
    )
    si = drain_inst.ins.sync_info
    waits = list(si.on_wait) if si is not None else []
    if len(waits) > 1:
        si.on_wait = [waits[0]]
        drain_inst.ins.sync_info = si
        for w in waits[1:]:
            d2 = nc.sync.drain()
            si2 = d2.ins.sync_info
            if si2 is None:
                si2 = bass_rust.SyncInfo(on_wait=[w], on_update=[])
            else:
                si2.on_wait = [w]
            d2.ins.sync_info = si2
    nc.all_engine_barrier()
    assert self.sems is not None
    popped = nc._tile_sem_poison_stack.pop()
    assert popped is self._sem_poison
    nc.clear_and_free_semaphores(list(self.sems.allocated().values()))
    nc.all_engine_barrier()


def _apply_tile_patch():
    global _PATCHED
    if not _PATCHED:
        tile.TileContext._drain_and_barrier = _patched_drain_and_barrier
        _PATCHED = True


def _split_multi_waits(nc):
    """Post-pass: the external walrus accepts only 1 sync wait per
    instruction (2 for EventSemaphore). Tile emits more. Hoist extra waits
    onto same-engine no-ops inserted just before. For compute engines this
    is identical semantics (the engine blocks either way). For DMA triggers
    it turns queue-side waits into SP-side blocking, which is safe in this
    forward-dataflow single-block program (every wait's producer precedes
    the trigger in the scheduled stream); CoreSim re-validates no-deadlock."""
    for f in nc.m.functions:
        for bb in f.blocks:
            new = []
            for ins in bb.instructions:
                si = ins.sync_info
                if si is not None:
                    cap = 2 if isinstance(ins, mybir.InstEventSemaphore) else 1
                    waits = list(si.on_wait)
                    if len(waits) > cap:
                        for w in waits[:-cap]:
                            nop = mybir.InstNoOp(
                                name=nc.get_next_instruction_name(),
                                engine=ins.engine,
                                sync_info=bass_rust.SyncInfo(
                                    on_wait=[w], on_update=[]
                                ),
                                bass_nofuse=True,
                            )
                            nc.register_instruction(nop, overwrite=True)
                            new.append(nop)
                        si.on_wait = waits[-cap:]
                        ins.sync_info = si
                new.append(ins)
            bb.instructions = new


# ---------------------------------------------------------------------------
_PROGRAM = None


def build_program():
    global _PROGRAM
    if _PROGRAM is not None:
        return _PROGRAM
    _apply_tile_patch()
    Exp = mybir.ActivationFunctionType.Exp
    Log = mybir.ActivationFunctionType.Ln

    nc = bass.Bass()
    xt_d = nc.declare_dram_parameter("xt", [D, T], F32R, isOutput=False)
    wq_d = nc.declare_dram_parameter("wq", [D, CW], F32R, isOutput=False)
    wk_d = nc.declare_dram_parameter("wk", [D, CW], F32R, isOutput=False)
    wv_d = nc.declare_dram_parameter("wv", [D, CW], F32R, isOutput=False)
    wo_d = nc.declare_dram_parameter("wo", [CW, D], F32R, isOutput=False)
    mask2_d = nc.declare_dram_parameter("mask2", [P, 4 * QW], F32R, isOutput=False)
    out_d = nc.declare_dram_parameter("out", [T, D], F32R, isOutput=True)

    with tile.TileContext(nc) as tc:
        from contextlib import ExitStack

        ctx = ExitStack()
        with ctx:
            consts = ctx.enter_context(tc.tile_pool(name="consts", bufs=1))
            xt_pool = ctx.enter_context(tc.tile_pool(name="xt", bufs=FC))
            qk_pool = ctx.enter_context(tc.tile_pool(name="qk", bufs=1))
            v_pool = ctx.enter_context(tc.tile_pool(name="v", bufs=1))
            exp_pool = ctx.enter_context(tc.tile_pool(name="exp", bufs=8))
            ctxt_pool = ctx.enter_context(tc.tile_pool(name="ctxt", bufs=1))
            rcp_pool = ctx.enter_context(tc.tile_pool(name="rcp", bufs=2))
            ob_pool = ctx.enter_context(tc.tile_pool(name="ob", bufs=3))

            ps_s = ctx.enter_context(tc.tile_pool(name="ps_s", bufs=2, space="PSUM"))
            ps_ctx = ctx.enter_context(
                tc.tile_pool(name="ps_ctx", bufs=1, space="PSUM")
            )
            ps_px = ctx.enter_context(tc.tile_pool(name="ps_px", bufs=2, space="PSUM"))

            # ---- constants ----
            wq_sb = consts.tile([P, FC, CW], F32R, tag="wq")
            wk_sb = consts.tile([P, FC, CW], F32R, tag="wk")
            wv_sb = consts.tile([P, FC, CW], F32R, tag="wv")
            wo_sb = [
                consts.tile([P, D], F32R, tag=f"wo{p}", name=f"wo_sb{p}")
                for p in range(NPAIR)
            ]
            mask2_sb = consts.tile([P, 4 * QW], F32R, tag="mask2")
            ones_sb = consts.tile([97, DK], F32, tag="ones")
            # DMA triggers spread across engine queues so the ~0.6us
            # per-trigger cost parallelizes and x^T lands ASAP
            # HAM warm-up: ~5us of dummy matmuls on a memset tile flip the
            # PE clock gate to 8/8 during the input-DMA window, so the real
            # projection prologue runs at 2.4 GHz instead of 1.2
            warm_sb = consts.tile([P, QW], F32R, tag="warm")
            nc.vector.memset(warm_sb, 0.0)
            for wi in range(12):
                wps = ps_px.tile([P, QW], F32, tag="px", name=f"warm{wi}")
                nc.tensor.matmul(
                    wps, lhsT=warm_sb[:, 0:P], rhs=warm_sb, start=True, stop=True
                )

            # x^T chunks lead the trigger queues (they gate the projection
            # accumulation); weights follow, except wq which gates MM #1
            nc.sync.dma_start(out=wq_sb, in_=wq_d.rearrange("(f p) c -> p f c", p=P))
            nc.vector.memset(ones_sb, 1.0)
            xts = [None] * FC
            dma_engines = [nc.sync, nc.scalar, nc.gpsimd]
            for fc in range(FC):
                xt_t = xt_pool.tile([P, T], F32R, tag="xt", name=f"xt{fc}")
                dma_engines[fc % 3].dma_start(
                    out=xt_t, in_=xt_d[fc * P : (fc + 1) * P, :]
                )
                xts[fc] = xt_t
            nc.scalar.dma_start(out=wk_sb, in_=wk_d.rearrange("(f p) c -> p f c", p=P))
            nc.gpsimd.dma_start(out=mask2_sb, in_=mask2_d[:, :])
            nc.gpsimd.dma_start(out=wv_sb, in_=wv_d.rearrange("(f p) c -> p f c", p=P))
            for p in range(NPAIR):
                nc.sync.dma_start(out=wo_sb[p], in_=wo_d[p * P : (p + 1) * P, :])

            qt = [
                qk_pool.tile([P, T], F32R, tag=f"qt{p}", name=f"qt{p}")
                for p in range(NPAIR)
            ]
            kt = [
                qk_pool.tile([P, T], F32R, tag=f"kt{p}", name=f"kt{p}")
                for p in range(NPAIR)
            ]
            v_sb = [
                v_pool.tile([P, KC, 2 * 65], F32R, tag=f"v{p}", name=f"v_sb{p}")
                for p in range(NPAIR)
            ]
            ctxt = [
                ctxt_pool.tile([P, T], F32R, tag=f"c{p}", name=f"ctxt{p}")
                for p in range(NPAIR)
            ]

            def emit_qk_one(rc, pair, which):
                w_sb, dst = (wq_sb, qt[pair]) if which == 0 else (wk_sb, kt[pair])
                ps = ps_px.tile([P, QW], F32, tag="px", name=f"qk{rc}{pair}{which}")
                for fc in range(FC):
                    nc.tensor.matmul(
                        ps,
                        lhsT=w_sb[:, fc, pair * P : (pair + 1) * P],
                        rhs=xts[fc][:, rc * QW : (rc + 1) * QW],
                        start=(fc == 0),
                        stop=(fc == FC - 1),
                    )
                nc.vector.tensor_copy(dst[:, rc * QW : (rc + 1) * QW], ps)

            def emit_v_kc(kc):
                ps = ps_px.tile([P, CW], F32, tag="px", name=f"vps{kc}")
                for fc in range(FC):
                    nc.tensor.matmul(
                        ps,
                        lhsT=xts[fc][:, kc * P : (kc + 1) * P],
                        rhs=wv_sb[:, fc, :],
                        start=(fc == 0),
                        stop=(fc == FC - 1),
                    )
                for pair in range(NPAIR):
                    # both heads' 64 cols in one strided copy: dst he-step 65
                    nc.vector.tensor_copy(
                        v_sb[pair][:, kc, :]
                        .rearrange("p (he x) -> p he x", he=2)[:, :, 0:DK],
                        ps[:, pair * P : (pair + 1) * P]
                        .rearrange("p (he x) -> p he x", he=2),
                    )

            def emit_ones_cols():
                # mask2 stair(0) cols 256:272 are all-ones
                src = mask2_sb[:, 256 : 256 + KC].rearrange("p (c o) -> p c o", o=1)
                for pair in range(NPAIR):
                    nc.vector.tensor_copy(v_sb[pair][:, :, DK : DK + 1], src)
                    nc.vector.tensor_copy(
                        v_sb[pair][:, :, 65 + DK : 65 + DK + 1], src
                    )

            def emit_drain_pair(qn, pair, ctx_ps, den97):
                # drain unnormalized ctx to bf16 SBUF and the denominator
                # rows to 32-aligned partitions of the shared den tile, so
                # the PSUM accumulators free up immediately
                for he in range(2):
                    nc.vector.tensor_copy(
                        ctxt[pair][
                            he * DK : (he + 1) * DK, qn * QW : (qn + 1) * QW
                        ],
                        ctx_ps[he][0:DK, :],
                    )
                    r = 32 * (2 * pair + he)
                    nc.vector.tensor_copy(
                        den97[r : r + 1, :], ctx_ps[he][DK : DK + 1, :]
                    )

            rcp_tiles = {}

            def emit_norm_lnexp(qn, den97):
                # 1/den = exp(-ln(den)) for all 4 heads in two ScalarE calls
                # over rows 0/32/64/96 at once
                lnd = rcp_pool.tile([97, QW], F32, tag="lnd", name=f"lnd{qn}")
                rcp97 = rcp_pool.tile([97, QW], F32, tag="rcp", name=f"rcp{qn}")
                nc.scalar.activation(out=lnd, in_=den97, func=Log)
                nc.scalar.activation(out=rcp97, in_=lnd, func=Exp, scale=-1.0)
                rcp_tiles[qn] = rcp97

            def emit_norm_bcmul(qn, pair, he):
                # K=1 matmul broadcasts 1/den across the head's 64 partitions;
                # in-place multiply normalizes the bf16 ctx block
                rcp97 = rcp_tiles[qn]
                r = 32 * (2 * pair + he)
                bc = ps_px.tile([DK, QW], F32, tag="px", name=f"bc{qn}{pair}{he}")
                nc.tensor.matmul(
                    bc,
                    lhsT=ones_sb[r : r + 1, :],
                    rhs=rcp97[r : r + 1, :],
                    start=True,
                    stop=True,
                    tile_position=(r, 0),
                )
                dst = ctxt[pair][he * DK : (he + 1) * DK, qn * QW : (qn + 1) * QW]
                nc.vector.tensor_mul(dst, dst, bc)

            def emit_outproj_rc(qn, i2):
                rc = qn * 4 + i2
                ob = ob_pool.tile([P, D], F32R, tag="ob", name=f"ob{rc}")
                for c2 in range(2):
                    ps = ps_px.tile([P, QW], F32, tag="px", name=f"o{rc}{c2}")
                    for pair in range(NPAIR):
                        nc.tensor.matmul(
                            ps,
                            lhsT=ctxt[pair][:, rc * P : (rc + 1) * P],
                            rhs=wo_sb[pair][:, c2 * QW : (c2 + 1) * QW],
                            start=(pair == 0),
                            stop=(pair == NPAIR - 1),
                        )
                    nc.vector.tensor_copy(ob[:, c2 * QW : (c2 + 1) * QW], ps)
                nc.sync.dma_start(out=out_d[rc * P : (rc + 1) * P, :], in_=ob)

            # ---- flat software-pipelined attention stream ----
            # The ScalarE exp stream is the bottleneck; S matmuls run one
            # group ahead of the ctx matmuls so exp(g+1) never waits on PE
            # work that is queued behind ctx(g). Projections for qn+1, the
            # normalization, and the output projection are interleaved as
            # "filler" slices between attention groups so the PE/DVE queues
            # stay dense (HAM-warm) without starving the exp pipeline.
            from collections import deque

            filler = deque()
            emitted = set()

            def pop_one():
                key, fn = filler.popleft()
                fn()
                if key:
                    emitted.add(key)

            def pops(groups_remaining):
                n = -(-len(filler) // max(1, groups_remaining))
                for _ in range(min(n, 3)):
                    if filler:
                        pop_one()

            def need(*keys):
                # drain fillers until all producer keys have been emitted:
                # Tile derives dependencies from trace order, so a consumer
                # must never be traced before its producer
                for k in keys:
                    while k not in emitted:
                        pop_one()

            # prologue: only what flat[0] = (qn0, pair0, kc 0/1) needs;
            # pair1's QK and V kc2/3 flow through the filler
            for w in range(2):
                emit_qk_one(0, 0, w)
                emitted.add(("qk", 0, 0, w))
            for kc in range(2):
                emit_v_kc(kc)
                emitted.add(("v", kc))
            emit_ones_cols()
            for w in range(2):
                filler.append(
                    (("qk", 0, 1, w), lambda w=w: emit_qk_one(0, 1, w))
                )
            for kc in (2, 3):
                filler.append((("v", kc), lambda kc=kc: emit_v_kc(kc)))

            flat = []
            for qn in range(QN):
                for pair in range(NPAIR):
                    nkc = 4 * (qn + 1)
                    for kc2 in range(0, nkc, 2):
                        flat.append((qn, pair, kc2, nkc))

            s_tiles = {}

            def emit_S(i):
                qn, pair, kc2, nkc = flat[i]
                need(*[("qk", r, pair, w) for r in range(qn + 1) for w in range(2)])
                sp = {
                    he: ps_s.tile([P, 2 * QW], F32, tag="s", name=f"s{i}{he}")
                    for he in range(2)
                }
                s_tiles[i] = sp
                # he0/he1 interleaved: the K=64 matmuls land on PE row groups
                # 0-63 / 64-127 back-to-back (concurrent row tiling)
                for half in range(2):
                    kc = kc2 + half
                    for he in range(2):
                        nc.tensor.matmul(
                            sp[he][:, half * QW : (half + 1) * QW],
                            lhsT=kt[pair][
                                he * DK : (he + 1) * DK, kc * P : (kc + 1) * P
                            ],
                            rhs=qt[pair][
                                he * DK : (he + 1) * DK, qn * QW : (qn + 1) * QW
                            ],
                            start=True,
                            stop=True,
                        )

            ctx_tiles = {}
            den_tiles = {}
            mask_alt = [0]
            emit_S(0)
            for i, (qn, pair, kc2, nkc) in enumerate(flat):
                if kc2 == 0:
                    ctx_tiles[(qn, pair)] = {
                        he: ps_ctx.tile(
                            [65, QW], F32, tag=f"x{he}", name=f"ctx{qn}{pair}{he}"
                        )
                        for he in range(2)
                    }
                    if pair == 0:
                        den_tiles[qn] = rcp_pool.tile(
                            [97, QW], F32, tag="den", name=f"den{qn}"
                        )
                        if qn + 1 < QN:
                            for p2 in range(NPAIR):
                                for w in range(2):
                                    filler.append(
                                        (
                                            ("qk", qn + 1, p2, w),
                                            lambda rc=qn + 1, p=p2, w=w: (
                                                emit_qk_one(rc, p, w)
                                            ),
                                        )
                                    )
                            for kc in range(4 * (qn + 1), 4 * (qn + 1) + 4):
                                filler.append(
                                    (("v", kc), lambda kc=kc: emit_v_kc(kc))
                                )
                ctx_ps = ctx_tiles[(qn, pair)]
                sp = s_tiles.pop(i)
                es = {}
                for he in range(2):
                    e = exp_pool.tile([P, 2 * QW], F32R, tag="exp", name=f"e{i}{he}")
                    nc.scalar.activation(out=e, in_=sp[he], func=Exp, scale=0.125)
                    dg = 0 if kc2 == 4 * qn else (1 if kc2 == 4 * qn + 2 else -1)
                    if dg >= 0:
                        eng = nc.vector
                        if GPSIMD_MASKS:
                            mask_alt[0] ^= 1
                            if mask_alt[0]:
                                eng = nc.gpsimd
                        eng.tensor_mul(
                            e, e, mask2_sb[:, dg * 2 * QW : (dg + 1) * 2 * QW]
                        )
                    es[he] = e
                if i + 1 < len(flat):
                    emit_S(i + 1)
                need(("v", kc2), ("v", kc2 + 1))
                for half in range(2):
                    kc = kc2 + half
                    for he in range(2):
                        nc.tensor.matmul(
                            ctx_ps[he],
                            lhsT=v_sb[pair][:, kc, he * 65 : he * 65 + 65],
                            rhs=es[he][:, half * QW : (half + 1) * QW],
                            start=(kc == 0),
                            stop=(kc == nkc - 1),
                        )
                if kc2 == nkc - 2:
                    emit_drain_pair(qn, pair, ctx_ps, den_tiles[qn])
                    del ctx_tiles[(qn, pair)]
                    if pair == NPAIR - 1:
                        d97 = den_tiles[qn]
                        filler.append(
                            (None, lambda qn=qn, d=d97: emit_norm_lnexp(qn, d))
                        )
                        for p2 in range(NPAIR):
                            for he in range(2):
                                filler.append(
                                    (
                                        None,
                                        lambda qn=qn, p=p2, he=he: (
                                            emit_norm_bcmul(qn, p, he)
                                        ),
                                    )
                                )
                        for i2 in range(4):
                            filler.append(
                                (
                                    None,
                                    lambda qn=qn, i2=i2: emit_outproj_rc(qn, i2),
                                )
                            )
                pops(len(flat) - i)
            while filler:
                pop_one()

    _split_multi_waits(nc)
    _PROGRAM = nc
    return nc


def _make_mask2():
    # mask2[:, 512j:512j+512] = stair(j): [k, q] = 1.0 iff q >= 128j + k
    k = np.arange(P)[:, None]
    q = np.arange(QW)[None, :]
    blocks = [(q >= 128 * j + k).astype(np.float32) for j in range(4)]
    return np.concatenate(blocks, axis=1)


def make_in_maps(x, Wq, Wk, Wv, Wo):
    import ml_dtypes

    nd = ml_dtypes.bfloat16 if F32R == BF16 else np.float32
    x = np.asarray(x, dtype=np.float32)
    mask2 = _make_mask2().astype(nd)
    Wq, Wk, Wv, Wo = (np.asarray(w, dtype=np.float32) for w in (Wq, Wk, Wv, Wo))
    xts = [np.ascontiguousarray(x[b].T).astype(nd) for b in range(B)]  # [1024,2048]
    in_maps = []
    for c in range(NCORES):
        b, q4 = divmod(c, NCORES // B)
        cols = slice(q4 * CW, (q4 + 1) * CW)
        in_maps.append(
            {
                "xt": xts[b],
                "wq": np.ascontiguousarray(Wq[:, cols]).astype(nd),
                "wk": np.ascontiguousarray(Wk[:, cols]).astype(nd),
                "wv": np.ascontiguousarray(Wv[:, cols]).astype(nd),
                "wo": np.ascontiguousarray(Wo[cols, :]).astype(nd),
                "mask2": mask2,
            }
        )
    return in_maps


def reduce_outputs(results):
    """Sum the per-core bf16 partials (4 cores per batch) in f64."""
    out = np.zeros((B, T, D), dtype=np.float64)
    for c in range(NCORES):
        b = c // (NCORES // B)
        out[b] += np.asarray(results[c]["out"], dtype=np.float64)
    return out.astype(np.float32)


def kernel(x, Wq, Wk, Wv, Wo):
    from concourse.bass_utils import run_bass_kernel_spmd

    nc = build_program()
    in_maps = make_in_maps(x, Wq, Wk, Wv, Wo)
    res = run_bass_kernel_spmd(nc, in_maps, core_ids=list(range(NCORES)))
    return reduce_outputs(res.results)


if __name__ == "__main__":
    rng = np.random.default_rng(0)
    s = 1.0 / np.sqrt(D)
    ins = {
        "x": rng.standard_normal((B, T, D)).astype(np.float32),
        "Wq": (rng.standard_normal((D, D)) * s).astype(np.float32),
        "Wk": (rng.standard_normal((D, D)) * s).astype(np.float32),
        "Wv": (rng.standard_normal((D, D)) * s).astype(np.float32),
        "Wo": (rng.standard_normal((D, D)) * (1.0 / np.sqrt(D))).astype(np.float32),
    }
    out = kernel(**ins)
    print("out", out.shape, out.dtype, float(np.abs(out).max()))


# revision 23
# speedup vs baseline: 1.0245x; 1.0245x over previous
"""Trainium2 Bass kernel for 16-head causal MHA (B=2, T=2048, D=1024, fp32 I/O).

Sharding: core c owns batch c//4 and head-quad c%4 (heads 4q..4q+3, as two
head-pairs). It computes Q/K/V projections for its 256 q/k/v dims, causal
attention for its 4 heads, and a partial output [2048, 1024] (bf16); the host
sums the 4 partials per batch in f64. One batch per core halves the partial-
output drain (PSUM->SBUF casts + DMA) and the x^T input DMA vs 2-batch cores.

Per-core device program, matmul inputs bf16 (2 cols/PE-cycle + fast weight
load), fp32 PSUM accumulation. The schedule is emitted as a 4-stage software
pipeline (QK rc-block + V chunk quad + attention qn + norm/out-proj qn) so
the ScalarE exp stream - the bottleneck engine at ~92us - starts a few us in
and stays saturated, while PE/DVE/DMA work (projections, out-proj, casts)
fills the gaps and keeps the PE HAM-warm:
  - Q^T, K^T = W.T @ x^T (weights stationary, N=512 moving blocks)
  - V natural = x @ Wv with a ones column per head (denominator for free)
  - attention in S^T layout per head-pair: the two heads' K=64 contractions
    sit at PE base partitions 0/64 (row-tiled, concurrent); exp on ScalarE
    with the 1/sqrt(dk) scale folded in; causality = skipping fully-masked
    blocks + one [128,1024] staircase multiply per diagonal 2-chunk group
    (mask2 packs the 4 staircase patterns contiguously)
  - normalization: 1/den via DVE reciprocal_approx_fast read straight from
    the PSUM denominator row; broadcast across partitions via a K=1 matmul;
    applied by a fused tensor_mul that also performs the PSUM->SBUF move
  - partial out per 128-query chunk: two accumulating K=128 matmuls (one per
    head-pair), cast to bf16, DMA'd per 1024-col row block.

Infrastructure: the external walrus allows only ONE sync wait per
instruction; a post-pass hoists extra waits onto single-wait no-ops and the
TileContext closing drain is split into a chain of single-wait drains.
"""

import numpy as np

import bass_rust
from bass_rust import ScopedClock
import concourse.bass as bass
import concourse.mybir as mybir
import concourse.tile as tile

F32 = mybir.dt.float32
BF16 = mybir.dt.bfloat16
F32R = BF16
B, T, D = 2, 2048, 1024
NCORES = 8
P = 128          # partitions / feature chunk
FC = D // P      # 8 feature chunks
QW = 512         # query block width (PSUM bank)
QN = T // QW     # 4 query blocks
KC = T // P      # 16 key chunks
DK = 64
CW = 256         # q/k/v dims per core (4 heads x 64)
NPAIR = 2        # head-pairs per core

# Set True to offload half the diagonal-mask multiplies to GpSimd.
GPSIMD_MASKS = False

# ---------------------------------------------------------------------------
# TileContext drain fix: the external walrus in this container allows only ONE
# sync wait per instruction, but Tile's closing drain packs one wait per active
# proc. Split it into a chain of single-wait drains (same semantics).
_PATCHED = False


def _patched_drain_and_barrier(self, tick_clock, wait_clock):
    nc = self.nc
    drain_inst = nc.sync.drain()
    wait_clock.add_sem_waits(
        drain_inst.ins, ScopedClock({None: tick_clock.global_clock})
    )
    si = drain_inst.ins.sync_info
    waits = list(si.on_wait) if si is not None else []
    if len(waits) > 1:
        si.on_wait = [waits[0]]
        drain_inst.ins.sync_info = si
        for w in waits[1:]:
            d2 = nc.sync.drain()
            si2 = d2.ins.sync_info
            if si2 is None:
                si2 = bass_rust.SyncInfo(on_wait=[w], on_update=[])
            else:
                si2.on_wait = [w]
            d2.ins.sync_info = si2
    nc.all_engine_barrier()
    assert self.sems is not None
    popped = nc._tile_sem_poison_stack.pop()
    assert popped is self._sem_poison
    nc.clear_and_free_semaphores(list(self.sems.allocated().values()))
    nc.all_engine_barrier()


def _apply_tile_patch():
    global _PATCHED
    if not _PATCHED:
        tile.TileContext._drain_and_barrier = _patched_drain_and_barrier
        _PATCHED = True


def _split_multi_waits(nc):
    """Post-pass: the external walrus accepts only 1 sync wait per
    instruction (2 for EventSemaphore). Tile emits more. Hoist extra waits
    onto same-engine no-ops inserted just before. For compute engines this
    is identical semantics (the engine blocks either way). For DMA triggers
    it turns queue-side waits into SP-side blocking, which is safe in this
    forward-dataflow single-block program (every wait's producer precedes
    the trigger in the scheduled stream); CoreSim re-validates no-deadlock."""
    for f in nc.m.functions:
        for bb in f.blocks:
            new = []
            for ins in bb.instructions:
                si = ins.sync_info
                if si is not None:
                    cap = 2 if isinstance(ins, mybir.InstEventSemaphore) else 1
                    waits = list(si.on_wait)
                    if len(waits) > cap:
                        for w in waits[:-cap]:
                            nop = mybir.InstNoOp(
                                name=nc.get_next_instruction_name(),
                                engine=ins.engine,
                                sync_info=bass_rust.SyncInfo(
                                    on_wait=[w], on_update=[]
                                ),
                                bass_nofuse=True,
                            )
                            nc.register_instruction(nop, overwrite=True)
                            new.append(nop)
                        si.on_wait = waits[-cap:]
                        ins.sync_info = si
                new.append(ins)
            bb.instructions = new


# ---------------------------------------------------------------------------
_PROGRAM = None


def build_program():
    global _PROGRAM
    if _PROGRAM is not None:
        return _PROGRAM
    _apply_tile_patch()
    Exp = mybir.ActivationFunctionType.Exp
    Log = mybir.ActivationFunctionType.Ln

    nc = bass.Bass()
    xt_d = nc.declare_dram_parameter("xt", [D, T], F32R, isOutput=False)
    wq_d = nc.declare_dram_parameter("wq", [D, CW], F32R, isOutput=False)
    wk_d = nc.declare_dram_parameter("wk", [D, CW], F32R, isOutput=False)
    wv_d = nc.declare_dram_parameter("wv", [D, CW], F32R, isOutput=False)
    wo_d = nc.declare_dram_parameter("wo", [CW, D], F32R, isOutput=False)
    mask2_d = nc.declare_dram_parameter("mask2", [P, 4 * QW], F32R, isOutput=False)
    out_d = nc.declare_dram_parameter("out", [T, D], F32R, isOutput=True)

    with tile.TileContext(nc) as tc:
        from contextlib import ExitStack

        ctx = ExitStack()
        with ctx:
            consts = ctx.enter_context(tc.tile_pool(name="consts", bufs=1))
            xt_pool = ctx.enter_context(tc.tile_pool(name="xt", bufs=FC))
            qk_pool = ctx.enter_context(tc.tile_pool(name="qk", bufs=1))
            v_pool = ctx.enter_context(tc.tile_pool(name="v", bufs=1))
            exp_pool = ctx.enter_context(tc.tile_pool(name="exp", bufs=8))
            ctxt_pool = ctx.enter_context(tc.tile_pool(name="ctxt", bufs=1))
            rcp_pool = ctx.enter_context(tc.tile_pool(name="rcp", bufs=2))
            ob_pool = ctx.enter_context(tc.tile_pool(name="ob", bufs=3))

            ps_s = ctx.enter_context(tc.tile_pool(name="ps_s", bufs=2, space="PSUM"))
            ps_ctx = ctx.enter_context(
                tc.tile_pool(name="ps_ctx", bufs=1, space="PSUM")
            )
            ps_px = ctx.enter_context(tc.tile_pool(name="ps_px", bufs=2, space="PSUM"))

            # ---- constants ----
            wq_sb = consts.tile([P, FC, CW], F32R, tag="wq")
            wk_sb = consts.tile([P, FC, CW], F32R, tag="wk")
            wv_sb = consts.tile([P, FC, CW], F32R, tag="wv")
            wo_sb = [
                consts.tile([P, D], F32R, tag=f"wo{p}", name=f"wo_sb{p}")
                for p in range(NPAIR)
            ]
            mask2_sb = consts.tile([P, 4 * QW], F32R, tag="mask2")
            ones_sb = consts.tile([97, DK], F32, tag="ones")
            # DMA triggers spread across engine queues so the ~0.6us
            # per-trigger cost parallelizes and x^T lands ASAP
            # HAM warm-up: ~5us of dummy matmuls on a memset tile flip the
            # PE clock gate to 8/8 during the input-DMA window, so the real
            # projection prologue runs at 2.4 GHz instead of 1.2
            warm_sb = consts.tile([P, QW], F32R, tag="warm")
            nc.vector.memset(warm_sb, 0.0)
            for wi in range(12):
                wps = ps_px.tile([P, QW], F32, tag="px", name=f"warm{wi}")
                nc.tensor.matmul(
                    wps, lhsT=warm_sb[:, 0:P], rhs=warm_sb, start=True, stop=True
                )

            # x^T chunks lead the trigger queues (they gate the projection
            # accumulation); weights follow, except wq which gates MM #1
            nc.sync.dma_start(out=wq_sb, in_=wq_d.rearrange("(f p) c -> p f c", p=P))
            nc.vector.memset(ones_sb, 1.0)
            xts = [None] * FC
            qmap = [nc.sync, nc.scalar, nc.gpsimd, nc.sync, nc.scalar,
                    nc.gpsimd, nc.scalar, nc.gpsimd]
            for fc in range(FC):
                xt_t = xt_pool.tile([P, T], F32R, tag="xt", name=f"xt{fc}")
                qmap[fc].dma_start(
                    out=xt_t, in_=xt_d[fc * P : (fc + 1) * P, :]
                )
                xts[fc] = xt_t
            nc.scalar.dma_start(out=wk_sb, in_=wk_d.rearrange("(f p) c -> p f c", p=P))
            nc.gpsimd.dma_start(out=mask2_sb, in_=mask2_d[:, :])
            nc.gpsimd.dma_start(out=wv_sb, in_=wv_d.rearrange("(f p) c -> p f c", p=P))
            for p in range(NPAIR):
                nc.sync.dma_start(out=wo_sb[p], in_=wo_d[p * P : (p + 1) * P, :])

            qt = [
                qk_pool.tile([P, T], F32R, tag=f"qt{p}", name=f"qt{p}")
                for p in range(NPAIR)
            ]
            kt = [
                qk_pool.tile([P, T], F32R, tag=f"kt{p}", name=f"kt{p}")
                for p in range(NPAIR)
            ]
            v_sb = [
                v_pool.tile([P, KC, 2 * 65], F32R, tag=f"v{p}", name=f"v_sb{p}")
                for p in range(NPAIR)
            ]
            ctxt = [
                ctxt_pool.tile([P, T], F32R, tag=f"c{p}", name=f"ctxt{p}")
                for p in range(NPAIR)
            ]

            def emit_qk_one(rc, pair, which):
                w_sb, dst = (wq_sb, qt[pair]) if which == 0 else (wk_sb, kt[pair])
                ps = ps_px.tile([P, QW], F32, tag="px", name=f"qk{rc}{pair}{which}")
                for fc in range(FC):
                    nc.tensor.matmul(
                        ps,
                        lhsT=w_sb[:, fc, pair * P : (pair + 1) * P],
                        rhs=xts[fc][:, rc * QW : (rc + 1) * QW],
                        start=(fc == 0),
                        stop=(fc == FC - 1),
                    )
                nc.vector.tensor_copy(dst[:, rc * QW : (rc + 1) * QW], ps)

            def emit_v_kc(kc):
                ps = ps_px.tile([P, CW], F32, tag="px", name=f"vps{kc}")
                for fc in range(FC):
                    nc.tensor.matmul(
                        ps,
                        lhsT=xts[fc][:, kc * P : (kc + 1) * P],
                        rhs=wv_sb[:, fc, :],
                        start=(fc == 0),
                        stop=(fc == FC - 1),
                    )
                for pair in range(NPAIR):
                    # both heads' 64 cols in one strided copy: dst he-step 65
                    nc.vector.tensor_copy(
                        v_sb[pair][:, kc, :]
                        .rearrange("p (he x) -> p he x", he=2)[:, :, 0:DK],
                        ps[:, pair * P : (pair + 1) * P]
                        .rearrange("p (he x) -> p he x", he=2),
                    )

            def emit_ones_cols():
                # mask2 stair(0) cols 256:272 are all-ones
                src = mask2_sb[:, 256 : 256 + KC].rearrange("p (c o) -> p c o", o=1)
                for pair in range(NPAIR):
                    nc.vector.tensor_copy(v_sb[pair][:, :, DK : DK + 1], src)
                    nc.vector.tensor_copy(
                        v_sb[pair][:, :, 65 + DK : 65 + DK + 1], src
                    )

            def emit_drain_pair(qn, pair, ctx_ps, den97):
                # drain unnormalized ctx to bf16 SBUF and the denominator
                # rows to 32-aligned partitions of the shared den tile, so
                # the PSUM accumulators free up immediately
                for he in range(2):
                    nc.vector.tensor_copy(
                        ctxt[pair][
                            he * DK : (he + 1) * DK, qn * QW : (qn + 1) * QW
                        ],
                        ctx_ps[he][0:DK, :],
                    )
                    r = 32 * (2 * pair + he)
                    nc.vector.tensor_copy(
                        den97[r : r + 1, :], ctx_ps[he][DK : DK + 1, :]
                    )

            rcp_tiles = {}

            def emit_norm_lnexp(qn, den97):
                # 1/den = exp(-ln(den)) for all 4 heads in two ScalarE calls
                # over rows 0/32/64/96 at once
                lnd = rcp_pool.tile([97, QW], F32, tag="lnd", name=f"lnd{qn}")
                rcp97 = rcp_pool.tile([97, QW], F32, tag="rcp", name=f"rcp{qn}")
                nc.scalar.activation(out=lnd, in_=den97, func=Log)
                nc.scalar.activation(out=rcp97, in_=lnd, func=Exp, scale=-1.0)
                rcp_tiles[qn] = rcp97

            def emit_norm_bcmul(qn, pair, he):
                # K=1 matmul broadcasts 1/den across the head's 64 partitions;
                # in-place multiply normalizes the bf16 ctx block
                rcp97 = rcp_tiles[qn]
                r = 32 * (2 * pair + he)
                bc = ps_px.tile([DK, QW], F32, tag="px", name=f"bc{qn}{pair}{he}")
                nc.tensor.matmul(
                    bc,
                    lhsT=ones_sb[r : r + 1, :],
                    rhs=rcp97[r : r + 1, :],
                    start=True,
                    stop=True,
                    tile_position=(r, 0),
                )
                dst = ctxt[pair][he * DK : (he + 1) * DK, qn * QW : (qn + 1) * QW]
                nc.vector.tensor_mul(dst, dst, bc)

            def emit_outproj_rc(qn, i2):
                rc = qn * 4 + i2
                ob = ob_pool.tile([P, D], F32R, tag="ob", name=f"ob{rc}")
                for c2 in range(2):
                    ps = ps_px.tile([P, QW], F32, tag="px", name=f"o{rc}{c2}")
                    for pair in range(NPAIR):
                        nc.tensor.matmul(
                            ps,
                            lhsT=ctxt[pair][:, rc * P : (rc + 1) * P],
                            rhs=wo_sb[pair][:, c2 * QW : (c2 + 1) * QW],
                            start=(pair == 0),
                            stop=(pair == NPAIR - 1),
                        )
                    nc.vector.tensor_copy(ob[:, c2 * QW : (c2 + 1) * QW], ps)
                nc.sync.dma_start(out=out_d[rc * P : (rc + 1) * P, :], in_=ob)

            # ---- flat software-pipelined attention stream ----
            # The ScalarE exp stream is the bottleneck; S matmuls run one
            # group ahead of the ctx matmuls so exp(g+1) never waits on PE
            # work that is queued behind ctx(g). Projections for qn+1, the
            # normalization, and the output projection are interleaved as
            # "filler" slices between attention groups so the PE/DVE queues
            # stay dense (HAM-warm) without starving the exp pipeline.
            from collections import deque

            projq = deque()   # QK/V projection slices: gate later attention
            slackq = deque()  # norm + out-proj slices: no downstream deadline
            emitted = set()

            def pop_one():
                if projq:
                    key, fn = projq.popleft()
                    fn()
                    emitted.add(key)
                elif slackq:
                    slackq.popleft()()

            def pops():
                n = 1 if projq or len(slackq) < 12 else 2
                for _ in range(n):
                    if projq or slackq:
                        pop_one()

            def need(*keys):
                # drain proj fillers until all producer keys are emitted:
                # Tile derives dependencies from trace order, so a consumer
                # must never be traced before its producer
                for k in keys:
                    while k not in emitted:
                        key, fn = projq.popleft()
                        fn()
                        emitted.add(key)

            # prologue: only what flat[0] = (qn0, pair0, kc 0/1) needs;
            # pair1's QK and V kc2/3 flow through the filler
            for w in range(2):
                emit_qk_one(0, 0, w)
                emitted.add(("qk", 0, 0, w))
            for kc in range(2):
                emit_v_kc(kc)
                emitted.add(("v", kc))
            emit_ones_cols()
            for w in range(2):
                projq.append(
                    (("qk", 0, 1, w), lambda w=w: emit_qk_one(0, 1, w))
                )
            for kc in (2, 3):
                projq.append((("v", kc), lambda kc=kc: emit_v_kc(kc)))

            flat = []
            for qn in range(QN):
                for pair in range(NPAIR):
                    nkc = 4 * (qn + 1)
                    for kc2 in range(0, nkc, 2):
                        flat.append((qn, pair, kc2, nkc))

            s_tiles = {}

            def emit_S(i):
                qn, pair, kc2, nkc = flat[i]
                need(*[("qk", r, pair, w) for r in range(qn + 1) for w in range(2)])
                sp = {
                    he: ps_s.tile([P, 2 * QW], F32, tag="s", name=f"s{i}{he}")
                    for he in range(2)
                }
                s_tiles[i] = sp
                # he0/he1 interleaved: the K=64 matmuls land on PE row groups
                # 0-63 / 64-127 back-to-back (concurrent row tiling)
                for half in range(2):
                    kc = kc2 + half
                    for he in range(2):
                        nc.tensor.matmul(
                            sp[he][:, half * QW : (half + 1) * QW],
                            lhsT=kt[pair][
                                he * DK : (he + 1) * DK, kc * P : (kc + 1) * P
                            ],
                            rhs=qt[pair][
                                he * DK : (he + 1) * DK, qn * QW : (qn + 1) * QW
                            ],
                            start=True,
                            stop=True,
                        )

            ctx_tiles = {}
            den_tiles = {}
            mask_alt = [0]
            emit_S(0)
            for i, (qn, pair, kc2, nkc) in enumerate(flat):
                if kc2 == 0:
                    ctx_tiles[(qn, pair)] = {
                        he: ps_ctx.tile(
                            [65, QW], F32, tag=f"x{he}", name=f"ctx{qn}{pair}{he}"
                        )
                        for he in range(2)
                    }
                    if pair == 0:
                        den_tiles[qn] = rcp_pool.tile(
                            [97, QW], F32, tag="den", name=f"den{qn}"
                        )
                        if qn + 1 < QN:
                            for p2 in range(NPAIR):
                                for w in range(2):
                                    projq.append(
                                        (
                                            ("qk", qn + 1, p2, w),
                                            lambda rc=qn + 1, p=p2, w=w: (
                                                emit_qk_one(rc, p, w)
                                            ),
                                        )
                                    )
                            for kc in range(4 * (qn + 1), 4 * (qn + 1) + 4):
                                projq.append(
                                    (("v", kc), lambda kc=kc: emit_v_kc(kc))
                                )
                ctx_ps = ctx_tiles[(qn, pair)]
                sp = s_tiles.pop(i)
                es = {}
                for he in range(2):
                    e = exp_pool.tile([P, 2 * QW], F32R, tag="exp", name=f"e{i}{he}")
                    nc.scalar.activation(out=e, in_=sp[he], func=Exp, scale=0.125)
                    dg = 0 if kc2 == 4 * qn else (1 if kc2 == 4 * qn + 2 else -1)
                    if dg >= 0:
                        eng = nc.vector
                        if GPSIMD_MASKS:
                            mask_alt[0] ^= 1
                            if mask_alt[0]:
                                eng = nc.gpsimd
                        eng.tensor_mul(
                            e, e, mask2_sb[:, dg * 2 * QW : (dg + 1) * 2 * QW]
                        )
                    es[he] = e
                if i + 1 < len(flat):
                    emit_S(i + 1)
                need(("v", kc2), ("v", kc2 + 1))
                for half in range(2):
                    kc = kc2 + half
                    for he in range(2):
                        nc.tensor.matmul(
                            ctx_ps[he],
                            lhsT=v_sb[pair][:, kc, he * 65 : he * 65 + 65],
                            rhs=es[he][:, half * QW : (half + 1) * QW],
                            start=(kc == 0),
                            stop=(kc == nkc - 1),
                        )
                if kc2 == nkc - 2:
                    emit_drain_pair(qn, pair, ctx_ps, den_tiles[qn])
                    del ctx_tiles[(qn, pair)]
                    if pair == NPAIR - 1:
                        d97 = den_tiles[qn]
                        slackq.append(
                            lambda qn=qn, d=d97: emit_norm_lnexp(qn, d)
                        )
                        for p2 in range(NPAIR):
                            for he in range(2):
                                slackq.append(
                                    lambda qn=qn, p=p2, he=he: (
                                        emit_norm_bcmul(qn, p, he)
                                    )
                                )
                        for i2 in range(4):
                            slackq.append(
                                lambda qn=qn, i2=i2: emit_outproj_rc(qn, i2)
                            )
                pops()
            while projq or slackq:
                pop_one()

    _split_multi_waits(nc)
    _PROGRAM = nc
    return nc


def _make_mask2():
    # mask2[:, 512j:512j+512] = stair(j): [k, q] = 1.0 iff q >= 128j + k
    k = np.arange(P)[:, None]
    q = np.arange(QW)[None, :]
    blocks = [(q >= 128 * j + k).astype(np.float32) for j in range(4)]
    return np.concatenate(blocks, axis=1)


def make_in_maps(x, Wq, Wk, Wv, Wo):
    import ml_dtypes

    nd = ml_dtypes.bfloat16 if F32R == BF16 else np.float32
    x = np.asarray(x, dtype=np.float32)
    mask2 = _make_mask2().astype(nd)
    Wq, Wk, Wv, Wo = (np.asarray(w, dtype=np.float32) for w in (Wq, Wk, Wv, Wo))
    xts = [np.ascontiguousarray(x[b].T).astype(nd) for b in range(B)]  # [1024,2048]
    in_maps = []
    for c in range(NCORES):
        b, q4 = divmod(c, NCORES // B)
        cols = slice(q4 * CW, (q4 + 1) * CW)
        in_maps.append(
            {
                "xt": xts[b],
                "wq": np.ascontiguousarray(Wq[:, cols]).astype(nd),
                "wk": np.ascontiguousarray(Wk[:, cols]).astype(nd),
                "wv": np.ascontiguousarray(Wv[:, cols]).astype(nd),
                "wo": np.ascontiguousarray(Wo[cols, :]).astype(nd),
                "mask2": mask2,
            }
        )
    return in_maps


def reduce_outputs(results):
    """Sum the per-core bf16 partials (4 cores per batch) in f64."""
    out = np.zeros((B, T, D), dtype=np.float64)
    for c in range(NCORES):
        b = c // (NCORES // B)
        out[b] += np.asarray(results[c]["out"], dtype=np.float64)
    return out.astype(np.float32)


def kernel(x, Wq, Wk, Wv, Wo):
    from concourse.bass_utils import run_bass_kernel_spmd

    nc = build_program()
    in_maps = make_in_maps(x, Wq, Wk, Wv, Wo)
    res = run_bass_kernel_spmd(nc, in_maps, core_ids=list(range(NCORES)))
    return reduce_outputs(res.results)


if __name__ == "__main__":
    rng = np.random.default_rng(0)
    s = 1.0 / np.sqrt(D)
    ins = {
        "x": rng.standard_normal((B, T, D)).astype(np.float32),
        "Wq": (rng.standard_normal((D, D)) * s).astype(np.float32),
        "Wk": (rng.standard_normal((D, D)) * s).astype(np.float32),
        "Wv": (rng.standard_normal((D, D)) * s).astype(np.float32),
        "Wo": (rng.standard_normal((D, D)) * (1.0 / np.sqrt(D))).astype(np.float32),
    }
    out = kernel(**ins)
    print("out", out.shape, out.dtype, float(np.abs(out).max()))


# revision 24
# speedup vs baseline: 1.0725x; 1.0469x over previous
"""Trainium2 Bass kernel for 16-head causal MHA (B=2, T=2048, D=1024, fp32 I/O).

Sharding: core c owns batch c//4 and head-quad c%4 (heads 4q..4q+3, as two
head-pairs). It computes Q/K/V projections for its 256 q/k/v dims, causal
attention for its 4 heads, and a partial output [2048, 1024] (bf16); the host
sums the 4 partials per batch in f64. One batch per core halves the partial-
output drain (PSUM->SBUF casts + DMA) and the x^T input DMA vs 2-batch cores.

Per-core device program, matmul inputs bf16 (2 cols/PE-cycle + fast weight
load), fp32 PSUM accumulation. The schedule is emitted as a 4-stage software
pipeline (QK rc-block + V chunk quad + attention qn + norm/out-proj qn) so
the ScalarE exp stream - the bottleneck engine at ~92us - starts a few us in
and stays saturated, while PE/DVE/DMA work (projections, out-proj, casts)
fills the gaps and keeps the PE HAM-warm:
  - Q^T, K^T = W.T @ x^T (weights stationary, N=512 moving blocks)
  - V natural = x @ Wv with a ones column per head (denominator for free)
  - attention in S^T layout per head-pair: the two heads' K=64 contractions
    sit at PE base partitions 0/64 (row-tiled, concurrent); exp on ScalarE
    with the 1/sqrt(dk) scale folded in; causality = skipping fully-masked
    blocks + one [128,1024] staircase multiply per diagonal 2-chunk group
    (mask2 packs the 4 staircase patterns contiguously)
  - normalization: 1/den via DVE reciprocal_approx_fast read straight from
    the PSUM denominator row; broadcast across partitions via a K=1 matmul;
    applied by a fused tensor_mul that also performs the PSUM->SBUF move
  - partial out per 128-query chunk: two accumulating K=128 matmuls (one per
    head-pair), cast to bf16, DMA'd per 1024-col row block.

Infrastructure: the external walrus allows only ONE sync wait per
instruction; a post-pass hoists extra waits onto single-wait no-ops and the
TileContext closing drain is split into a chain of single-wait drains.
"""

import numpy as np

import bass_rust
from bass_rust import ScopedClock
import concourse.bass as bass
import concourse.mybir as mybir
import concourse.tile as tile

F32 = mybir.dt.float32
BF16 = mybir.dt.bfloat16
F32R = BF16
B, T, D = 2, 2048, 1024
NCORES = 8
P = 128          # partitions / feature chunk
FC = D // P      # 8 feature chunks
QW = 512         # query block width (PSUM bank)
QN = T // QW     # 4 query blocks
KC = T // P      # 16 key chunks
DK = 64
CW = 256         # q/k/v dims per core (4 heads x 64)
NPAIR = 2        # head-pairs per core

# Set True to offload half the diagonal-mask multiplies to GpSimd.
GPSIMD_MASKS = False

# ---------------------------------------------------------------------------
# TileContext drain fix: the external walrus in this container allows only ONE
# sync wait per instruction, but Tile's closing drain packs one wait per active
# proc. Split it into a chain of single-wait drains (same semantics).
_PATCHED = False


def _patched_drain_and_barrier(self, tick_clock, wait_clock):
    nc = self.nc
    drain_inst = nc.sync.drain()
    wait_clock.add_sem_waits(
        drain_inst.ins, ScopedClock({None: tick_clock.global_clock})
    )
    si = drain_inst.ins.sync_info
    waits = list(si.on_wait) if si is not None else []
    if len(waits) > 1:
        si.on_wait = [waits[0]]
        drain_inst.ins.sync_info = si
        for w in waits[1:]:
            d2 = nc.sync.drain()
            si2 = d2.ins.sync_info
            if si2 is None:
                si2 = bass_rust.SyncInfo(on_wait=[w], on_update=[])
            else:
                si2.on_wait = [w]
            d2.ins.sync_info = si2
    nc.all_engine_barrier()
    assert self.sems is not None
    popped = nc._tile_sem_poison_stack.pop()
    assert popped is self._sem_poison
    nc.clear_and_free_semaphores(list(self.sems.allocated().values()))
    nc.all_engine_barrier()


def _apply_tile_patch():
    global _PATCHED
    if not _PATCHED:
        tile.TileContext._drain_and_barrier = _patched_drain_and_barrier
        _PATCHED = True


def _split_multi_waits(nc):
    """Post-pass: the external walrus accepts only 1 sync wait per
    instruction (2 for EventSemaphore). Tile emits more. Hoist extra waits
    onto same-engine no-ops inserted just before. For compute engines this
    is identical semantics (the engine blocks either way). For DMA triggers
    it turns queue-side waits into SP-side blocking, which is safe in this
    forward-dataflow single-block program (every wait's producer precedes
    the trigger in the scheduled stream); CoreSim re-validates no-deadlock."""
    for f in nc.m.functions:
        for bb in f.blocks:
            new = []
            for ins in bb.instructions:
                si = ins.sync_info
                if si is not None:
                    cap = 2 if isinstance(ins, mybir.InstEventSemaphore) else 1
                    waits = list(si.on_wait)
                    if len(waits) > cap:
                        for w in waits[:-cap]:
                            nop = mybir.InstNoOp(
                                name=nc.get_next_instruction_name(),
                                engine=ins.engine,
                                sync_info=bass_rust.SyncInfo(
                                    on_wait=[w], on_update=[]
                                ),
                                bass_nofuse=True,
                            )
                            nc.register_instruction(nop, overwrite=True)
                            new.append(nop)
                        si.on_wait = waits[-cap:]
                        ins.sync_info = si
                new.append(ins)
            bb.instructions = new


# ---------------------------------------------------------------------------
_PROGRAM = None


def build_program():
    global _PROGRAM
    if _PROGRAM is not None:
        return _PROGRAM
    _apply_tile_patch()
    Exp = mybir.ActivationFunctionType.Exp
    Log = mybir.ActivationFunctionType.Ln

    nc = bass.Bass()
    xt_d = nc.declare_dram_parameter("xt", [D, T], F32R, isOutput=False)
    wq_d = nc.declare_dram_parameter("wq", [D, CW], F32R, isOutput=False)
    wk_d = nc.declare_dram_parameter("wk", [D, CW], F32R, isOutput=False)
    wv_d = nc.declare_dram_parameter("wv", [D, CW], F32R, isOutput=False)
    wo_d = nc.declare_dram_parameter("wo", [CW, D], F32R, isOutput=False)
    mask2_d = nc.declare_dram_parameter("mask2", [P, 4 * QW], F32R, isOutput=False)
    out_d = nc.declare_dram_parameter("out", [T, D], F32R, isOutput=True)

    with tile.TileContext(nc) as tc:
        from contextlib import ExitStack

        ctx = ExitStack()
        with ctx:
            consts = ctx.enter_context(tc.tile_pool(name="consts", bufs=1))
            xt_pool = ctx.enter_context(tc.tile_pool(name="xt", bufs=1))
            qk_pool = ctx.enter_context(tc.tile_pool(name="qk", bufs=1))
            v_pool = ctx.enter_context(tc.tile_pool(name="v", bufs=1))
            exp_pool = ctx.enter_context(tc.tile_pool(name="exp", bufs=8))
            ctxt_pool = ctx.enter_context(tc.tile_pool(name="ctxt", bufs=1))
            rcp_pool = ctx.enter_context(tc.tile_pool(name="rcp", bufs=2))
            ob_pool = ctx.enter_context(tc.tile_pool(name="ob", bufs=3))

            ps_s = ctx.enter_context(tc.tile_pool(name="ps_s", bufs=2, space="PSUM"))
            ps_ctx = ctx.enter_context(
                tc.tile_pool(name="ps_ctx", bufs=1, space="PSUM")
            )
            ps_px = ctx.enter_context(tc.tile_pool(name="ps_px", bufs=2, space="PSUM"))

            # ---- constants ----
            wq_sb = consts.tile([P, FC, CW], F32R, tag="wq")
            wk_sb = consts.tile([P, FC, CW], F32R, tag="wk")
            wv_sb = consts.tile([P, FC, CW], F32R, tag="wv")
            wo_sb = [
                consts.tile([P, D], F32R, tag=f"wo{p}", name=f"wo_sb{p}")
                for p in range(NPAIR)
            ]
            mask2_sb = consts.tile([P, 4 * QW], F32R, tag="mask2")
            ones_sb = consts.tile([97, DK], F32, tag="ones")
            # DMA triggers spread across engine queues so the ~0.6us
            # per-trigger cost parallelizes and x^T lands ASAP
            # HAM warm-up: ~5us of dummy matmuls on a memset tile flip the
            # PE clock gate to 8/8 during the input-DMA window, so the real
            # projection prologue runs at 2.4 GHz instead of 1.2
            warm_sb = consts.tile([P, QW], F32R, tag="warm")
            nc.vector.memset(warm_sb, 0.0)
            for wi in range(12):
                wps = ps_px.tile([P, QW], F32, tag="px", name=f"warm{wi}")
                nc.tensor.matmul(
                    wps, lhsT=warm_sb[:, 0:P], rhs=warm_sb, start=True, stop=True
                )

            # x^T arrives by query/key block: qn0's attention needs only
            # tokens 0-511, so rc0 (split across two queues) lands in ~7us
            # and the first exp fires ~25us earlier than a full-x wait
            nc.vector.memset(ones_sb, 1.0)
            xt_all = xt_pool.tile([P, FC, T], F32R, tag="xt", name="xt_all")

            def xt_rearr(fclo, fchi, clo, chi):
                return xt_d[fclo * P : fchi * P, clo:chi].rearrange(
                    "(f p) c -> p f c", p=P
                )

            nc.sync.dma_start(
                out=xt_all[:, 0:4, 0:QW], in_=xt_rearr(0, 4, 0, QW)
            )
            nc.scalar.dma_start(out=wq_sb, in_=wq_d.rearrange("(f p) c -> p f c", p=P))
            nc.scalar.dma_start(
                out=xt_all[:, 4:FC, 0:QW], in_=xt_rearr(4, FC, 0, QW)
            )
            nc.gpsimd.dma_start(out=wk_sb, in_=wk_d.rearrange("(f p) c -> p f c", p=P))
            nc.gpsimd.dma_start(out=wv_sb, in_=wv_d.rearrange("(f p) c -> p f c", p=P))
            nc.sync.dma_start(
                out=xt_all[:, :, QW : 2 * QW], in_=xt_rearr(0, FC, QW, 2 * QW)
            )
            nc.scalar.dma_start(
                out=xt_all[:, :, 2 * QW : 3 * QW],
                in_=xt_rearr(0, FC, 2 * QW, 3 * QW),
            )
            nc.gpsimd.dma_start(out=mask2_sb, in_=mask2_d[:, :])
            nc.gpsimd.dma_start(
                out=xt_all[:, :, 3 * QW : 4 * QW],
                in_=xt_rearr(0, FC, 3 * QW, 4 * QW),
            )
            for p in range(NPAIR):
                nc.sync.dma_start(out=wo_sb[p], in_=wo_d[p * P : (p + 1) * P, :])
            xts = [xt_all[:, fc, :] for fc in range(FC)]

            qt = [
                qk_pool.tile([P, T], F32R, tag=f"qt{p}", name=f"qt{p}")
                for p in range(NPAIR)
            ]
            kt = [
                qk_pool.tile([P, T], F32R, tag=f"kt{p}", name=f"kt{p}")
                for p in range(NPAIR)
            ]
            v_sb = [
                v_pool.tile([P, KC, 2 * 65], F32R, tag=f"v{p}", name=f"v_sb{p}")
                for p in range(NPAIR)
            ]
            ctxt = [
                ctxt_pool.tile([P, T], F32R, tag=f"c{p}", name=f"ctxt{p}")
                for p in range(NPAIR)
            ]

            def emit_qk_one(rc, pair, which):
                w_sb, dst = (wq_sb, qt[pair]) if which == 0 else (wk_sb, kt[pair])
                ps = ps_px.tile([P, QW], F32, tag="px", name=f"qk{rc}{pair}{which}")
                for fc in range(FC):
                    nc.tensor.matmul(
                        ps,
                        lhsT=w_sb[:, fc, pair * P : (pair + 1) * P],
                        rhs=xts[fc][:, rc * QW : (rc + 1) * QW],
                        start=(fc == 0),
                        stop=(fc == FC - 1),
                    )
                nc.vector.tensor_copy(dst[:, rc * QW : (rc + 1) * QW], ps)

            def emit_v_kc(kc):
                ps = ps_px.tile([P, CW], F32, tag="px", name=f"vps{kc}")
                for fc in range(FC):
                    nc.tensor.matmul(
                        ps,
                        lhsT=xts[fc][:, kc * P : (kc + 1) * P],
                        rhs=wv_sb[:, fc, :],
                        start=(fc == 0),
                        stop=(fc == FC - 1),
                    )
                for pair in range(NPAIR):
                    # both heads' 64 cols in one strided copy: dst he-step 65
                    nc.vector.tensor_copy(
                        v_sb[pair][:, kc, :]
                        .rearrange("p (he x) -> p he x", he=2)[:, :, 0:DK],
                        ps[:, pair * P : (pair + 1) * P]
                        .rearrange("p (he x) -> p he x", he=2),
                    )

            def emit_ones_cols():
                # mask2 stair(0) cols 256:272 are all-ones
                src = mask2_sb[:, 256 : 256 + KC].rearrange("p (c o) -> p c o", o=1)
                for pair in range(NPAIR):
                    nc.vector.tensor_copy(v_sb[pair][:, :, DK : DK + 1], src)
                    nc.vector.tensor_copy(
                        v_sb[pair][:, :, 65 + DK : 65 + DK + 1], src
                    )

            def emit_drain_pair(qn, pair, ctx_ps, den97):
                # drain unnormalized ctx to bf16 SBUF and the denominator
                # rows to 32-aligned partitions of the shared den tile, so
                # the PSUM accumulators free up immediately
                for he in range(2):
                    nc.vector.tensor_copy(
                        ctxt[pair][
                            he * DK : (he + 1) * DK, qn * QW : (qn + 1) * QW
                        ],
                        ctx_ps[he][0:DK, :],
                    )
                    r = 32 * (2 * pair + he)
                    nc.vector.tensor_copy(
                        den97[r : r + 1, :], ctx_ps[he][DK : DK + 1, :]
                    )

            rcp_tiles = {}

            def emit_norm_lnexp(qn, den97):
                # 1/den = exp(-ln(den)) for all 4 heads in two ScalarE calls
                # over rows 0/32/64/96 at once
                lnd = rcp_pool.tile([97, QW], F32, tag="lnd", name=f"lnd{qn}")
                rcp97 = rcp_pool.tile([97, QW], F32, tag="rcp", name=f"rcp{qn}")
                nc.scalar.activation(out=lnd, in_=den97, func=Log)
                nc.scalar.activation(out=rcp97, in_=lnd, func=Exp, scale=-1.0)
                rcp_tiles[qn] = rcp97

            def emit_norm_bcmul(qn, pair, he):
                # K=1 matmul broadcasts 1/den across the head's 64 partitions;
                # in-place multiply normalizes the bf16 ctx block
                rcp97 = rcp_tiles[qn]
                r = 32 * (2 * pair + he)
                bc = ps_px.tile([DK, QW], F32, tag="px", name=f"bc{qn}{pair}{he}")
                nc.tensor.matmul(
                    bc,
                    lhsT=ones_sb[r : r + 1, :],
                    rhs=rcp97[r : r + 1, :],
                    start=True,
                    stop=True,
                    tile_position=(r, 0),
                )
                dst = ctxt[pair][he * DK : (he + 1) * DK, qn * QW : (qn + 1) * QW]
                nc.vector.tensor_mul(dst, dst, bc)

            def emit_outproj_rc(qn, i2):
                rc = qn * 4 + i2
                ob = ob_pool.tile([P, D], F32R, tag="ob", name=f"ob{rc}")
                for c2 in range(2):
                    ps = ps_px.tile([P, QW], F32, tag="px", name=f"o{rc}{c2}")
                    for pair in range(NPAIR):
                        nc.tensor.matmul(
                            ps,
                            lhsT=ctxt[pair][:, rc * P : (rc + 1) * P],
                            rhs=wo_sb[pair][:, c2 * QW : (c2 + 1) * QW],
                            start=(pair == 0),
                            stop=(pair == NPAIR - 1),
                        )
                    nc.vector.tensor_copy(ob[:, c2 * QW : (c2 + 1) * QW], ps)
                nc.sync.dma_start(out=out_d[rc * P : (rc + 1) * P, :], in_=ob)

            # ---- flat software-pipelined attention stream ----
            # The ScalarE exp stream is the bottleneck; S matmuls run one
            # group ahead of the ctx matmuls so exp(g+1) never waits on PE
            # work that is queued behind ctx(g). Projections for qn+1, the
            # normalization, and the output projection are interleaved as
            # "filler" slices between attention groups so the PE/DVE queues
            # stay dense (HAM-warm) without starving the exp pipeline.
            from collections import deque

            projq = deque()   # QK/V projection slices: gate later attention
            slackq = deque()  # norm + out-proj slices: no downstream deadline
            emitted = set()

            def pop_one():
                if projq:
                    key, fn = projq.popleft()
                    fn()
                    emitted.add(key)
                elif slackq:
                    slackq.popleft()()

            def pops():
                n = 1 if projq or len(slackq) < 12 else 2
                for _ in range(n):
                    if projq or slackq:
                        pop_one()

            def need(*keys):
                # drain proj fillers until all producer keys are emitted:
                # Tile derives dependencies from trace order, so a consumer
                # must never be traced before its producer
                for k in keys:
                    while k not in emitted:
                        key, fn = projq.popleft()
                        fn()
                        emitted.add(key)

            # prologue: only what flat[0] = (qn0, pair0, kc 0/1) needs;
            # pair1's QK and V kc2/3 flow through the filler
            for w in range(2):
                emit_qk_one(0, 0, w)
                emitted.add(("qk", 0, 0, w))
            for kc in range(2):
                emit_v_kc(kc)
                emitted.add(("v", kc))
            emit_ones_cols()
            for w in range(2):
                projq.append(
                    (("qk", 0, 1, w), lambda w=w: emit_qk_one(0, 1, w))
                )
            for kc in (2, 3):
                projq.append((("v", kc), lambda kc=kc: emit_v_kc(kc)))

            flat = []
            for qn in range(QN):
                for pair in range(NPAIR):
                    nkc = 4 * (qn + 1)
                    for kc2 in range(0, nkc, 2):
                        flat.append((qn, pair, kc2, nkc))

            s_tiles = {}

            def emit_S(i):
                qn, pair, kc2, nkc = flat[i]
                need(*[("qk", r, pair, w) for r in range(qn + 1) for w in range(2)])
                sp = {
                    he: ps_s.tile([P, 2 * QW], F32, tag="s", name=f"s{i}{he}")
                    for he in range(2)
                }
                s_tiles[i] = sp
                # he0/he1 interleaved: the K=64 matmuls land on PE row groups
                # 0-63 / 64-127 back-to-back (concurrent row tiling)
                for half in range(2):
                    kc = kc2 + half
                    for he in range(2):
                        nc.tensor.matmul(
                            sp[he][:, half * QW : (half + 1) * QW],
                            lhsT=kt[pair][
                                he * DK : (he + 1) * DK, kc * P : (kc + 1) * P
                            ],
                            rhs=qt[pair][
                                he * DK : (he + 1) * DK, qn * QW : (qn + 1) * QW
                            ],
                            start=True,
                            stop=True,
                        )

            ctx_tiles = {}
            den_tiles = {}
            mask_alt = [0]
            emit_S(0)
            for i, (qn, pair, kc2, nkc) in enumerate(flat):
                if kc2 == 0:
                    ctx_tiles[(qn, pair)] = {
                        he: ps_ctx.tile(
                            [65, QW], F32, tag=f"x{he}", name=f"ctx{qn}{pair}{he}"
                        )
                        for he in range(2)
                    }
                    if pair == 0:
                        den_tiles[qn] = rcp_pool.tile(
                            [97, QW], F32, tag="den", name=f"den{qn}"
                        )
                        if qn + 1 < QN:
                            for p2 in range(NPAIR):
                                for w in range(2):
                                    projq.append(
                                        (
                                            ("qk", qn + 1, p2, w),
                                            lambda rc=qn + 1, p=p2, w=w: (
                                                emit_qk_one(rc, p, w)
                                            ),
                                        )
                                    )
                            for kc in range(4 * (qn + 1), 4 * (qn + 1) + 4):
                                projq.append(
                                    (("v", kc), lambda kc=kc: emit_v_kc(kc))
                                )
                ctx_ps = ctx_tiles[(qn, pair)]
                sp = s_tiles.pop(i)
                es = {}
                for he in range(2):
                    e = exp_pool.tile([P, 2 * QW], F32R, tag="exp", name=f"e{i}{he}")
                    nc.scalar.activation(out=e, in_=sp[he], func=Exp, scale=0.125)
                    dg = 0 if kc2 == 4 * qn else (1 if kc2 == 4 * qn + 2 else -1)
                    if dg >= 0:
                        eng = nc.vector
                        if GPSIMD_MASKS:
                            mask_alt[0] ^= 1
                            if mask_alt[0]:
                                eng = nc.gpsimd
                        eng.tensor_mul(
                            e, e, mask2_sb[:, dg * 2 * QW : (dg + 1) * 2 * QW]
                        )
                    es[he] = e
                if i + 1 < len(flat):
                    emit_S(i + 1)
                need(("v", kc2), ("v", kc2 + 1))
                for half in range(2):
                    kc = kc2 + half
                    for he in range(2):
                        nc.tensor.matmul(
                            ctx_ps[he],
                            lhsT=v_sb[pair][:, kc, he * 65 : he * 65 + 65],
                            rhs=es[he][:, half * QW : (half + 1) * QW],
                            start=(kc == 0),
                            stop=(kc == nkc - 1),
                        )
                if kc2 == nkc - 2:
                    emit_drain_pair(qn, pair, ctx_ps, den_tiles[qn])
                    del ctx_tiles[(qn, pair)]
                    if pair == NPAIR - 1:
                        d97 = den_tiles[qn]
                        slackq.append(
                            lambda qn=qn, d=d97: emit_norm_lnexp(qn, d)
                        )
                        for p2 in range(NPAIR):
                            for he in range(2):
                                slackq.append(
                                    lambda qn=qn, p=p2, he=he: (
                                        emit_norm_bcmul(qn, p, he)
                                    )
                                )
                        for i2 in range(4):
                            slackq.append(
                                lambda qn=qn, i2=i2: emit_outproj_rc(qn, i2)
                            )
                pops()
            while projq or slackq:
                pop_one()

    _split_multi_waits(nc)
    _PROGRAM = nc
    return nc


def _make_mask2():
    # mask2[:, 512j:512j+512] = stair(j): [k, q] = 1.0 iff q >= 128j + k
    k = np.arange(P)[:, None]
    q = np.arange(QW)[None, :]
    blocks = [(q >= 128 * j + k).astype(np.float32) for j in range(4)]
    return np.concatenate(blocks, axis=1)


def make_in_maps(x, Wq, Wk, Wv, Wo):
    import ml_dtypes

    nd = ml_dtypes.bfloat16 if F32R == BF16 else np.float32
    x = np.asarray(x, dtype=np.float32)
    mask2 = _make_mask2().astype(nd)
    Wq, Wk, Wv, Wo = (np.asarray(w, dtype=np.float32) for w in (Wq, Wk, Wv, Wo))
    xts = [np.ascontiguousarray(x[b].T).astype(nd) for b in range(B)]  # [1024,2048]
    in_maps = []
    for c in range(NCORES):
        b, q4 = divmod(c, NCORES // B)
        cols = slice(q4 * CW, (q4 + 1) * CW)
        in_maps.append(
            {
                "xt": xts[b],
                "wq": np.ascontiguousarray(Wq[:, cols]).astype(nd),
                "wk": np.ascontiguousarray(Wk[:, cols]).astype(nd),
                "wv": np.ascontiguousarray(Wv[:, cols]).astype(nd),
                "wo": np.ascontiguousarray(Wo[cols, :]).astype(nd),
                "mask2": mask2,
            }
        )
    return in_maps


def reduce_outputs(results):
    """Sum the per-core bf16 partials (4 cores per batch) in f64."""
    out = np.zeros((B, T, D), dtype=np.float64)
    for c in range(NCORES):
        b = c // (NCORES // B)
        out[b] += np.asarray(results[c]["out"], dtype=np.float64)
    return out.astype(np.float32)


def kernel(x, Wq, Wk, Wv, Wo):
    from concourse.bass_utils import run_bass_kernel_spmd

    nc = build_program()
    in_maps = make_in_maps(x, Wq, Wk, Wv, Wo)
    res = run_bass_kernel_spmd(nc, in_maps, core_ids=list(range(NCORES)))
    return reduce_outputs(res.results)


if __name__ == "__main__":
    rng = np.random.default_rng(0)
    s = 1.0 / np.sqrt(D)
    ins = {
        "x": rng.standard_normal((B, T, D)).astype(np.float32),
        "Wq": (rng.standard_normal((D, D)) * s).astype(np.float32),
        "Wk": (rng.standard_normal((D, D)) * s).astype(np.float32),
        "Wv": (rng.standard_normal((D, D)) * s).astype(np.float32),
        "Wo": (rng.standard_normal((D, D)) * (1.0 / np.sqrt(D))).astype(np.float32),
    }
    out = kernel(**ins)
    print("out", out.shape, out.dtype, float(np.abs(out).max()))
